# revision 26
# baseline (speedup 1.0000x reference)
"""Trainium2 Bass kernel for ColumnAttention:
    out = softmax(query @ x^T + bias) @ x        (per batch sample)

Shapes: x [64, 576, 1024] f32, query [576, 1024] f32, bias [576, 576] f32.
Data-parallel over batch across 8 NeuronCores (8 samples per core).

v2: mm1 runs in fp8 e4m3 with DoubleRow perf mode (2 fp8 weights per PE
cell -> 256-deep contraction per matmul, ~2x bf16 rate; measured 122 ns
per 288-wide matmul = streaming roofline). query is pre-scaled by
QS=1024 into the e4m3 normal range; bias is pre-scaled by QS too, and
the exp activation applies scale=1/QS so the softmax sees the true
scores. mm2 stays bf16 (fp8 there would blow the 2e-2 error budget;
measured sim: mm1-fp8 = 8.9e-3, all-fp8 = 2.5e-2).

Per-core program:
  Samples are processed in PAIRS: the pair's key axis is 2*576 = 1152 =
  9*128, so every mm1 k-chunk has full 128 partitions.

  mm1:  scoresT[k, q] = sum_d x8[k, d] * q8T[d, q]   (DoubleRow fp8,
        4 chained matmuls of 256-deep contraction per 128-k-chunk)
  bias: DVE adds host-pretransposed QS-scaled [biasT; biasT] on drain
  exp:  ACT exp with scale=1/QS, bf16 out
  mm2:  out[q, d] = sum_k attnT[k, q]^T * x[k, d]    (bf16, per sample)
        q-tail (q 512:576, M=64): both samples' tails run CONCURRENTLY
        in disjoint PE column groups
  norm: DVE reciprocal; DVE per-partition tensor_scalar multiply on
        PSUM drain (moved off ACT: ACT was 71us busy in the bf16
        baseline), bf16 out (upcast to f32 on host).

  DMA queue plan (dedicated queues so loads never queue behind stores):
    sync:   qT halves, then xT fp8 per pair (3 kc-progressive chunks)
    scalar: qT halves, then x bf16 per pair (3 chunks)
    gpsimd: bias (2 chunks), out stores
  PSUM: psAB 3 banks (mm1 score chunks, rotating so the next chunk's
  matmuls overlap the previous chunk's DVE drain), psO 2x2 banks (mm2),
  psS 1 bank (softmax denominators via ones-column matmuls). Warmup
  matmuls on memset scratch borrow psO before its first real use.
"""

import sys

if "/opt/trn_rl_repo" not in sys.path:
    sys.path.insert(0, "/opt/trn_rl_repo")

import numpy as np
import ml_dtypes
from contextlib import ExitStack

B, NQ, D = 64, 576, 1024
NCORES = 8
BPC = B // NCORES      # samples per core
NPAIR = BPC // 2       # sample pairs per core

P = 128
NKC = 2 * NQ // P      # 9 pair k-chunks
NDC = D // P           # 8 d chunks
NQF = 4                # full q-chunks of 128 (576 = 4*128 + 64)
QTAIL = 512            # tail q offset (qs = 64)

QS = 1024.0            # query/bias pre-scale into e4m3 range
N_WARMUP = 16          # HAM warmup matmuls (N=128 each)

_BUILD_CACHE = {}


def build_program():
    """Build + compile the per-core Bass program. Returns the Bacc object."""
    if "nc" in _BUILD_CACHE:
        return _BUILD_CACHE["nc"]

    import concourse.mybir as mybir
    import concourse.tile as tile
    from concourse import bacc

    bf16 = mybir.dt.bfloat16
    f8 = mybir.dt.float8e4
    f32 = mybir.dt.float32
    AF = mybir.ActivationFunctionType
    DR = mybir.MatmulPerfMode.DoubleRow

    nc = bacc.Bacc(trn_type="TRN2", target_bir_lowering=False, debug=False)

    # x host-pre-rearranged to its SBUF layout [pair, p(row%128), kc, d]:
    # per-partition DMA runs become (hi-lo)*2048B contiguous (up to 18KB)
    # instead of 2048B - fat packets win DMA round-robin share, and x
    # delivery is the startup critical path
    xs = nc.dram_tensor("xs", [NPAIR, P, NKC, D], bf16, kind="ExternalInput")
    # kc-major pair-concatenated x^T: [pair, p(d%128), kc, dc, k%128] fp8
    xsT = nc.dram_tensor("xsT", [NPAIR, P, NKC, NDC, P], f8,
                         kind="ExternalInput")
    qT = nc.dram_tensor("qT", [P, NDC, NQ], f8, kind="ExternalInput")
    bTp = nc.dram_tensor("bTp", [P, NKC, NQ], bf16, kind="ExternalInput")
    out = nc.dram_tensor("out", [BPC, NQ, D], bf16, kind="ExternalOutput")

    with tile.TileContext(nc) as tc, ExitStack() as ctx:
        statics = ctx.enter_context(tc.tile_pool(name="statics", bufs=1))
        xpool = ctx.enter_context(tc.tile_pool(name="xpool", bufs=2))
        xtpool = ctx.enter_context(tc.tile_pool(name="xtpool", bufs=2))
        scpool = ctx.enter_context(tc.tile_pool(name="scpool", bufs=3))
        atpool = ctx.enter_context(tc.tile_pool(name="atpool", bufs=2))
        opool = ctx.enter_context(tc.tile_pool(name="opool", bufs=3))
        rpool = ctx.enter_context(tc.tile_pool(name="rpool", bufs=3))
        # PSUM: 3 + 4 + 1 = 8 banks
        psAB = ctx.enter_context(tc.tile_pool(name="psAB", bufs=3, space="PSUM"))
        psO = ctx.enter_context(tc.tile_pool(name="psO", bufs=2, space="PSUM"))
        psS = ctx.enter_context(tc.tile_pool(name="psS", bufs=1, space="PSUM"))

        # ---- HAM warmup: back-to-back matmuls on memset scratch keep the
        # PE clock-gate at 8/8 while the DMA queues fill SBUF. Borrow the
        # psO pool (its first real use comes ~10us later). ----
        wsc = statics.tile([P, 256], bf16)
        nc.gpsimd.memset(wsc, 0.0)
        for _ in range(N_WARMUP):
            pw = psO.tile([P, 128], f32, tag="po")
            nc.tensor.matmul(pw, wsc[:, 0:128], wsc[:, 128:256], start=True, stop=True)

        # ---- static params: qT split across the two load queues ----
        qT_sb = statics.tile([P, NDC, NQ], f8)
        qT_r = qT.ap()
        nc.sync.dma_start(out=qT_sb[:, 0:4, :], in_=qT_r[:, 0:4, :])
        nc.scalar.dma_start(out=qT_sb[:, 4:8, :], in_=qT_r[:, 4:8, :])
        bT_sb = statics.tile([P, NKC, NQ], bf16)
        nc.gpsimd.dma_start(out=bT_sb[:, 0:2, :], in_=bTp.ap()[:, 0:2, :])
        nc.gpsimd.dma_start(out=bT_sb[:, 2:9, :], in_=bTp.ap()[:, 2:9, :])
        ones_sb = statics.tile([P, 1], bf16)
        nc.gpsimd.memset(ones_sb, 1.0)

        def load_pair(pr):
            # xT fp8 on the sync queue (kc-progressive so mm1 kc0 starts
            # early); x bf16 on the scalar queue.
            xT_sb = xtpool.tile([P, NKC, NDC, P], f8, tag="xT")
            xT_r = xsT.ap()[pr]
            for lo, hi in ((0, 1), (1, 4), (4, 9)):
                nc.sync.dma_start(out=xT_sb[:, lo:hi], in_=xT_r[:, lo:hi])
            x_sb = xpool.tile([P, NKC, D], bf16, tag="x")
            x_r = xs.ap()[pr]
            for lo, hi in ((0, 3), (3, 6)):
                nc.scalar.dma_start(out=x_sb[:, lo:hi, :], in_=x_r[:, lo:hi, :])
            # the s1-phase chunks ride the sync queue: pair 0's mm2 starts
            # after x(0:6) only, trimming the startup critical path
            (nc.sync if pr == 0 else nc.scalar).dma_start(
                out=x_sb[:, 6:9, :], in_=x_r[:, 6:9, :])
            return x_sb, xT_sb

        def mm1_chunk(xT_sb, attnT, kc):
            """One pair k-chunk of scoresT + bias + exp (DoubleRow fp8)."""
            pa1 = psAB.tile([P, 512], f32, tag="pa")
            pa2 = psAB.tile([P, 512], f32, tag="pa")
            for dc in range(0, NDC, 2):
                w = xT_sb[:, kc, dc:dc + 2, :]
                st, sp = dc == 0, dc == NDC - 2
                nc.tensor.matmul(pa1[:, 0:288], w, qT_sb[:, dc:dc + 2, 0:288],
                                 start=st, stop=sp, perf_mode=DR)
                nc.tensor.matmul(pa2[:, 0:288], w, qT_sb[:, dc:dc + 2, 288:576],
                                 start=st, stop=sp, perf_mode=DR)
            sc = scpool.tile([P, NQ], f32, tag="sc")
            nc.vector.tensor_add(sc[:, 0:288], pa1[:, 0:288], bT_sb[:, kc, 0:288])
            nc.vector.tensor_add(sc[:, 288:576], pa2[:, 0:288], bT_sb[:, kc, 288:576])
            nc.scalar.activation(attnT[:, kc, :], sc, AF.Exp, scale=1.0 / QS)

        S_STEPS = [
            [(c, 0, P) for c in range(4)] + [(4, 0, 64)],
            [(4, 64, 64)] + [(c, 0, P) for c in range(5, 9)],
        ]

        def mm2_full(pr, s, qc, x_sb, attnT, ps_):
            """One (sample, full q-chunk of 128) of out = attn @ x + denom."""
            qb = qc * P
            po = psO.tile([P, 1024], f32, tag="po")
            steps = S_STEPS[s]
            for j, (c, pb, K) in enumerate(steps):
                w = attnT[pb:pb + K, c, qb:qb + P]
                st, sp = j == 0, j == len(steps) - 1
                nc.tensor.matmul(po[:, 0:512], w, x_sb[pb:pb + K, c, 0:512], start=st, stop=sp)
                nc.tensor.matmul(po[:, 512:1024], w, x_sb[pb:pb + K, c, 512:1024], start=st, stop=sp)
                nc.tensor.matmul(ps_[:, s:s + 1], w, ones_sb[pb:pb + K, :], start=st, stop=sp)
            r = rpool.tile([P, 1], f32, tag="r")
            nc.vector.reciprocal(r, ps_[:, s:s + 1])
            o = opool.tile([P, D], bf16, tag="o")
            if pr == NPAIR - 1:
                # last iteration has no mm1 exps, so ACT is idle: run norms
                # there so DVE serves only reciprocals (the s1-unit start
                # matmuls wait on them via the denominator-bank WAR), and
                # split stores across two queues for a faster final drain
                nc.scalar.activation(o, po, AF.Copy, scale=r)
                nc.gpsimd.dma_start(out=out.ap()[2 * pr + s, qb:qb + P, 0:512], in_=o[:, 0:512])
                nc.scalar.dma_start(out=out.ap()[2 * pr + s, qb:qb + P, 512:1024], in_=o[:, 512:1024])
            else:
                nc.vector.tensor_scalar_mul(o, po, r)
                nc.gpsimd.dma_start(out=out.ap()[2 * pr + s, qb:qb + P, :], in_=o)

        def mm2_tail(pr, x_sb, attnT, ps_):
            """Both samples' q-tail (q 512:576, M=64) concurrently: s0 in PE
            column groups 0-1 (PSUM partitions 0:64), s1 in groups 2-3."""
            po = psO.tile([P, 1024], f32, tag="po")
            for j in range(5):
                for s in range(2):
                    c, pb, K = S_STEPS[s][j]
                    w = attnT[pb:pb + K, c, QTAIL:NQ]
                    ob = 64 * s
                    st, sp = j == 0, j == 4
                    nc.tensor.matmul(po[ob:ob + 64, 0:512], w, x_sb[pb:pb + K, c, 0:512], start=st, stop=sp)
                    nc.tensor.matmul(po[ob:ob + 64, 512:1024], w, x_sb[pb:pb + K, c, 512:1024], start=st, stop=sp)
                    nc.tensor.matmul(ps_[ob:ob + 64, s:s + 1], w, ones_sb[pb:pb + K, :], start=st, stop=sp)
            r = rpool.tile([P, 1], f32, tag="r")
            nc.vector.reciprocal(r[0:64, :], ps_[0:64, 0:1])
            nc.vector.reciprocal(r[64:128, :], ps_[64:128, 1:2])
            o = opool.tile([P, D], bf16, tag="o")
            if pr == NPAIR - 1:
                nc.scalar.activation(o, po, AF.Copy, scale=r)
                nc.sync.dma_start(out=out.ap()[2 * pr, QTAIL:NQ, :], in_=o[0:64, :])
                nc.scalar.dma_start(out=out.ap()[2 * pr + 1, QTAIL:NQ, :], in_=o[64:128, :])
            else:
                nc.vector.tensor_scalar_mul(o, po, r)
                nc.gpsimd.dma_start(out=out.ap()[2 * pr, QTAIL:NQ, :], in_=o[0:64, :])
                nc.gpsimd.dma_start(out=out.ap()[2 * pr + 1, QTAIL:NQ, :], in_=o[64:128, :])

        # ---- prologue: pair 0 loads + mm1 chunks 0-4 (all that the
        # s0-major pair-0 units consume; chunks 5-8 interleave into the
        # s0 phase so mm2 starts ~6us earlier) ----
        x_cur, xT_cur = load_pair(0)
        attnT_cur = atpool.tile([P, NKC, NQ], bf16, tag="attnT")
        for kc in range(5):
            mm1_chunk(xT_cur, attnT_cur, kc)

        # ---- steady: mm2(pair p) interleaved with mm1(pair p+1) ----
        for pr in range(NPAIR):
            if pr + 1 < NPAIR:
                x_nxt, xT_nxt = load_pair(pr + 1)
                attnT_nxt = atpool.tile([P, NKC, NQ], bf16, tag="attnT")
            else:
                x_nxt = xT_nxt = attnT_nxt = None
            if pr == 0:
                units = ([(qc, 0) for qc in range(NQF)]
                         + [(qc, 1) for qc in range(NQF)] + [("tail", 0)])
                # own chunks 5-8 during the s0 phase, pair-1 chunks after
                jobs = ([(xT_cur, attnT_cur, kc) for kc in range(5, 9)]
                        + [(xT_nxt, attnT_nxt, kc) for kc in range(NKC)])
                sched = [[0], [1], [2], [3], [4, 5], [6, 7], [8, 9], [10, 11], [12]]
            else:
                units = [(qc, s) for qc in range(NQF) for s in range(2)] + [("tail", 0)]
                jobs = ([(xT_nxt, attnT_nxt, kc) for kc in range(NKC)]
                        if attnT_nxt is not None else [])
                sched = [[i] if i < len(jobs) else [] for i in range(len(units))]
            ps_cur = None
            for i, (qc, s) in enumerate(units):
                if qc == "tail":
                    ps_cur = psS.tile([P, 2], f32, tag="ps")
                    mm2_tail(pr, x_cur, attnT_cur, ps_cur)
                else:
                    if s == 0 or pr == 0:
                        ps_cur = psS.tile([P, 2], f32, tag="ps")
                    mm2_full(pr, s, qc, x_cur, attnT_cur, ps_cur)
                for j in sched[i]:
                    mm1_chunk(*jobs[j])
            x_cur, xT_cur, attnT_cur = x_nxt, xT_nxt, attnT_nxt

    nc.compile()
    _BUILD_CACHE["nc"] = nc
    return nc


def make_in_maps(x, query, bias):
    # qT: [D, NQ] -> [p, dc, q] with d = dc*128 + p, scaled by QS, fp8
    qT_np = np.ascontiguousarray(
        (query.astype(np.float32) * QS).T.reshape(NDC, P, NQ).transpose(1, 0, 2)
    ).astype(ml_dtypes.float8_e4m3)
    # bias: [q, k] -> biasT [k, q] scaled by QS, duplicated along the pair
    # k axis, then [p, kc, q] with k_pair = kc*128 + p
    bT = bias.T.astype(np.float32) * QS
    bTp_np = np.ascontiguousarray(
        np.concatenate([bT, bT], axis=0).reshape(NKC, P, NQ).transpose(1, 0, 2)
    ).astype(ml_dtypes.bfloat16)
    x_bf = x.astype(ml_dtypes.bfloat16)
    x_f8 = x.astype(ml_dtypes.float8_e4m3)
    # xT kc-major pair-concatenated:
    # [B, k, d] -> pairs [B/2, 1152, d] -> [pair, p(d%128), kc, dc, k%128]
    xp = x_f8.reshape(B // 2, 2 * NQ, D)                  # [pairs, kp, d]
    xp = xp.reshape(B // 2, NKC, P, NDC, P)               # [pr, kc, k1, dc, p]
    xT_np = np.ascontiguousarray(xp.transpose(0, 4, 1, 3, 2))  # [pr, p, kc, dc, k1]
    # x in SBUF layout: [pair, row%128, kc, d] with pair-row = kc*128 + p
    x_sr = np.ascontiguousarray(
        x_bf.reshape(B // 2, NKC, P, D).transpose(0, 2, 1, 3))
    in_maps = []
    for c in range(NCORES):
        in_maps.append({
            "xs": x_sr[c * NPAIR:(c + 1) * NPAIR],
            "xsT": xT_np[c * NPAIR:(c + 1) * NPAIR],
            "qT": qT_np,
            "bTp": bTp_np,
        })
    return in_maps


def kernel(x, query, bias):
    from concourse.bass_utils import run_bass_kernel_spmd

    nc = build_program()
    in_maps = make_in_maps(np.asarray(x), np.asarray(query), np.asarray(bias))
    res = run_bass_kernel_spmd(nc, in_maps, core_ids=list(range(NCORES)))
    return np.concatenate(
        [r["out"].astype(np.float32) for r in res.results], axis=0)


if __name__ == "__main__":
    rng = np.random.default_rng(0)
    x = rng.standard_normal((B, NQ, D), dtype=np.float32)
    q = rng.standard_normal((NQ, D), dtype=np.float32) / 32.0
    bias = 0.01 * rng.standard_normal((NQ, NQ), dtype=np.float32)
    o = kernel(x, q, bias)
    print(o.shape, o.dtype)


# revision 28
# speedup vs baseline: 1.0115x; 1.0115x over previous
"""Trainium2 Bass kernel for ColumnAttention:
    out = softmax(query @ x^T + bias) @ x        (per batch sample)

Shapes: x [64, 576, 1024] f32, query [576, 1024] f32, bias [576, 576] f32.
Data-parallel over batch across 8 NeuronCores (8 samples per core).

v2: mm1 runs in fp8 e4m3 with DoubleRow perf mode (2 fp8 weights per PE
cell -> 256-deep contraction per matmul, ~2x bf16 rate; measured 122 ns
per 288-wide matmul = streaming roofline). query is pre-scaled by
QS=1024 into the e4m3 normal range; bias is pre-scaled by QS too, and
the exp activation applies scale=1/QS so the softmax sees the true
scores. mm2 stays bf16 (fp8 there would blow the 2e-2 error budget;
measured sim: mm1-fp8 = 8.9e-3, all-fp8 = 2.5e-2).

Per-core program:
  Samples are processed in PAIRS: the pair's key axis is 2*576 = 1152 =
  9*128, so every mm1 k-chunk has full 128 partitions.

  mm1:  scoresT[k, q] = sum_d x8[k, d] * q8T[d, q]   (DoubleRow fp8,
        4 chained matmuls of 256-deep contraction per 128-k-chunk)
  bias: DVE adds host-pretransposed QS-scaled [biasT; biasT] on drain
  exp:  ACT exp with scale=1/QS, bf16 out
  mm2:  out[q, d] = sum_k attnT[k, q]^T * x[k, d]    (bf16, per sample)
        q-tail (q 512:576, M=64): both samples' tails run CONCURRENTLY
        in disjoint PE column groups
  norm: DVE reciprocal; DVE per-partition tensor_scalar multiply on
        PSUM drain (moved off ACT: ACT was 71us busy in the bf16
        baseline), bf16 out (upcast to f32 on host).

  DMA queue plan (dedicated queues so loads never queue behind stores):
    sync:   qT halves, then xT fp8 per pair (3 kc-progressive chunks)
    scalar: qT halves, then x bf16 per pair (3 chunks)
    gpsimd: bias (2 chunks), out stores
  PSUM: psAB 3 banks (mm1 score chunks, rotating so the next chunk's
  matmuls overlap the previous chunk's DVE drain), psO 2x2 banks (mm2),
  psS 1 bank (softmax denominators via ones-column matmuls). Warmup
  matmuls on memset scratch borrow psO before its first real use.
"""

import sys

if "/opt/trn_rl_repo" not in sys.path:
    sys.path.insert(0, "/opt/trn_rl_repo")

import numpy as np
import ml_dtypes
from contextlib import ExitStack

B, NQ, D = 64, 576, 1024
NCORES = 8
BPC = B // NCORES      # samples per core
NPAIR = BPC // 2       # sample pairs per core

P = 128
NKC = 2 * NQ // P      # 9 pair k-chunks
NDC = D // P           # 8 d chunks
NQF = 4                # full q-chunks of 128 (576 = 4*128 + 64)
QTAIL = 512            # tail q offset (qs = 64)

QS = 1024.0            # query/bias pre-scale into e4m3 range
N_WARMUP = 16          # HAM warmup matmuls (N=128 each)

_BUILD_CACHE = {}


def build_program():
    """Build + compile the per-core Bass program. Returns the Bacc object."""
    if "nc" in _BUILD_CACHE:
        return _BUILD_CACHE["nc"]

    import concourse.mybir as mybir
    import concourse.tile as tile
    from concourse import bacc

    bf16 = mybir.dt.bfloat16
    f8 = mybir.dt.float8e4
    f32 = mybir.dt.float32
    AF = mybir.ActivationFunctionType
    DR = mybir.MatmulPerfMode.DoubleRow

    nc = bacc.Bacc(trn_type="TRN2", target_bir_lowering=False, debug=False)

    xs = nc.dram_tensor("xs", [BPC, NQ, D], bf16, kind="ExternalInput")
    # kc-major pair-concatenated x^T: [pair, p(d%128), kc, dc, k%128] fp8
    xsT = nc.dram_tensor("xsT", [NPAIR, P, NKC, NDC, P], f8,
                         kind="ExternalInput")
    qT = nc.dram_tensor("qT", [P, NDC, NQ], f8, kind="ExternalInput")
    bTp = nc.dram_tensor("bTp", [P, NKC, NQ], bf16, kind="ExternalInput")
    out = nc.dram_tensor("out", [BPC, NQ, D], bf16, kind="ExternalOutput")

    with tile.TileContext(nc) as tc, ExitStack() as ctx:
        statics = ctx.enter_context(tc.tile_pool(name="statics", bufs=1))
        xpool = ctx.enter_context(tc.tile_pool(name="xpool", bufs=2))
        xtpool = ctx.enter_context(tc.tile_pool(name="xtpool", bufs=2))
        scpool = ctx.enter_context(tc.tile_pool(name="scpool", bufs=4))
        atpool = ctx.enter_context(tc.tile_pool(name="atpool", bufs=2))
        opool = ctx.enter_context(tc.tile_pool(name="opool", bufs=4))
        rpool = ctx.enter_context(tc.tile_pool(name="rpool", bufs=4))
        # PSUM: 3 + 4 + 1 = 8 banks
        psAB = ctx.enter_context(tc.tile_pool(name="psAB", bufs=3, space="PSUM"))
        psO = ctx.enter_context(tc.tile_pool(name="psO", bufs=2, space="PSUM"))
        psS = ctx.enter_context(tc.tile_pool(name="psS", bufs=1, space="PSUM"))

        # ---- HAM warmup: back-to-back matmuls on memset scratch keep the
        # PE clock-gate at 8/8 while the DMA queues fill SBUF. Borrow the
        # psO pool (its first real use comes ~10us later). ----
        wsc = statics.tile([P, 256], bf16)
        nc.gpsimd.memset(wsc, 0.0)
        for _ in range(N_WARMUP):
            pw = psO.tile([P, 128], f32, tag="po")
            nc.tensor.matmul(pw, wsc[:, 0:128], wsc[:, 128:256], start=True, stop=True)

        # ---- static params: qT split across the two load queues ----
        qT_sb = statics.tile([P, NDC, NQ], f8)
        qT_r = qT.ap()
        nc.sync.dma_start(out=qT_sb[:, 0:4, :], in_=qT_r[:, 0:4, :])
        nc.scalar.dma_start(out=qT_sb[:, 4:8, :], in_=qT_r[:, 4:8, :])
        bT_sb = statics.tile([P, NKC, NQ], bf16)
        nc.gpsimd.dma_start(out=bT_sb[:, 0:2, :], in_=bTp.ap()[:, 0:2, :])
        nc.gpsimd.dma_start(out=bT_sb[:, 2:9, :], in_=bTp.ap()[:, 2:9, :])
        ones_sb = statics.tile([P, 1], bf16)
        nc.gpsimd.memset(ones_sb, 1.0)

        def load_pair(pr):
            # xT fp8 on the sync queue (kc-progressive so mm1 kc0 starts
            # early); x bf16 on the scalar queue.
            xT_sb = xtpool.tile([P, NKC, NDC, P], f8, tag="xT")
            xT_r = xsT.ap()[pr]
            for lo, hi in ((0, 1), (1, 4), (4, 9)):
                nc.sync.dma_start(out=xT_sb[:, lo:hi], in_=xT_r[:, lo:hi])
            x_sb = xpool.tile([P, NKC, D], bf16, tag="x")
            x_r = (xs.ap()[2 * pr:2 * pr + 2].rearrange("b n d -> (b n) d")
                   .rearrange("(c p) d -> p c d", p=P))
            for lo, hi in ((0, 3), (3, 6)):
                nc.scalar.dma_start(out=x_sb[:, lo:hi, :], in_=x_r[:, lo:hi, :])
            # the s1-phase chunks ride the sync queue: pair 0's mm2 starts
            # after x(0:6) only, trimming the startup critical path
            (nc.sync if pr == 0 else nc.scalar).dma_start(
                out=x_sb[:, 6:9, :], in_=x_r[:, 6:9, :])
            return x_sb, xT_sb

        def mm1_chunk(xT_sb, attnT, kc):
            """One pair k-chunk of scoresT + bias + exp (DoubleRow fp8)."""
            pa1 = psAB.tile([P, 512], f32, tag="pa")
            pa2 = psAB.tile([P, 512], f32, tag="pa")
            for dc in range(0, NDC, 2):
                w = xT_sb[:, kc, dc:dc + 2, :]
                st, sp = dc == 0, dc == NDC - 2
                nc.tensor.matmul(pa1[:, 0:288], w, qT_sb[:, dc:dc + 2, 0:288],
                                 start=st, stop=sp, perf_mode=DR)
                nc.tensor.matmul(pa2[:, 0:288], w, qT_sb[:, dc:dc + 2, 288:576],
                                 start=st, stop=sp, perf_mode=DR)
            sc = scpool.tile([P, NQ], f32, tag="sc")
            nc.vector.tensor_add(sc[:, 0:288], pa1[:, 0:288], bT_sb[:, kc, 0:288])
            nc.vector.tensor_add(sc[:, 288:576], pa2[:, 0:288], bT_sb[:, kc, 288:576])
            nc.scalar.activation(attnT[:, kc, :], sc, AF.Exp, scale=1.0 / QS)

        S_STEPS = [
            [(c, 0, P) for c in range(4)] + [(4, 0, 64)],
            [(4, 64, 64)] + [(c, 0, P) for c in range(5, 9)],
        ]

        def mm2_full(pr, s, qc, x_sb, attnT, ps_):
            """One (sample, full q-chunk of 128) of out = attn @ x + denom."""
            qb = qc * P
            po = psO.tile([P, 1024], f32, tag="po")
            steps = S_STEPS[s]
            for j, (c, pb, K) in enumerate(steps):
                w = attnT[pb:pb + K, c, qb:qb + P]
                st, sp = j == 0, j == len(steps) - 1
                nc.tensor.matmul(po[:, 0:512], w, x_sb[pb:pb + K, c, 0:512], start=st, stop=sp)
                nc.tensor.matmul(po[:, 512:1024], w, x_sb[pb:pb + K, c, 512:1024], start=st, stop=sp)
                nc.tensor.matmul(ps_[:, s:s + 1], w, ones_sb[pb:pb + K, :], start=st, stop=sp)
            r = rpool.tile([P, 1], f32, tag="r")
            nc.vector.reciprocal(r, ps_[:, s:s + 1])
            o = opool.tile([P, D], bf16, tag="o")
            if pr == NPAIR - 1:
                # last iteration has no mm1 exps, so ACT is idle: run norms
                # there so DVE serves only reciprocals (the s1-unit start
                # matmuls wait on them via the denominator-bank WAR), and
                # split stores across two queues for a faster final drain
                nc.scalar.activation(o, po, AF.Copy, scale=r)
                nc.gpsimd.dma_start(out=out.ap()[2 * pr + s, qb:qb + P, 0:512], in_=o[:, 0:512])
                nc.scalar.dma_start(out=out.ap()[2 * pr + s, qb:qb + P, 512:1024], in_=o[:, 512:1024])
            else:
                nc.vector.tensor_scalar_mul(o, po, r)
                nc.gpsimd.dma_start(out=out.ap()[2 * pr + s, qb:qb + P, :], in_=o)

        def mm2_tail(pr, x_sb, attnT, ps_):
            """Both samples' q-tail (q 512:576, M=64) concurrently: s0 in PE
            column groups 0-1 (PSUM partitions 0:64), s1 in groups 2-3."""
            po = psO.tile([P, 1024], f32, tag="po")
            for j in range(5):
                for s in range(2):
                    c, pb, K = S_STEPS[s][j]
                    w = attnT[pb:pb + K, c, QTAIL:NQ]
                    ob = 64 * s
                    st, sp = j == 0, j == 4
                    nc.tensor.matmul(po[ob:ob + 64, 0:512], w, x_sb[pb:pb + K, c, 0:512], start=st, stop=sp)
                    nc.tensor.matmul(po[ob:ob + 64, 512:1024], w, x_sb[pb:pb + K, c, 512:1024], start=st, stop=sp)
                    nc.tensor.matmul(ps_[ob:ob + 64, s:s + 1], w, ones_sb[pb:pb + K, :], start=st, stop=sp)
            r = rpool.tile([P, 1], f32, tag="r")
            nc.vector.reciprocal(r[0:64, :], ps_[0:64, 0:1])
            nc.vector.reciprocal(r[64:128, :], ps_[64:128, 1:2])
            o = opool.tile([P, D], bf16, tag="o")
            if pr == NPAIR - 1:
                nc.scalar.activation(o, po, AF.Copy, scale=r)
                nc.sync.dma_start(out=out.ap()[2 * pr, QTAIL:NQ, :], in_=o[0:64, :])
                nc.scalar.dma_start(out=out.ap()[2 * pr + 1, QTAIL:NQ, :], in_=o[64:128, :])
            else:
                nc.vector.tensor_scalar_mul(o, po, r)
                nc.gpsimd.dma_start(out=out.ap()[2 * pr, QTAIL:NQ, :], in_=o[0:64, :])
                nc.gpsimd.dma_start(out=out.ap()[2 * pr + 1, QTAIL:NQ, :], in_=o[64:128, :])

        # ---- prologue: pair 0 loads + mm1 chunks 0-4 (all that the
        # s0-major pair-0 units consume; chunks 5-8 interleave into the
        # s0 phase so mm2 starts ~6us earlier) ----
        x_cur, xT_cur = load_pair(0)
        attnT_cur = atpool.tile([P, NKC, NQ], bf16, tag="attnT")
        for kc in range(5):
            mm1_chunk(xT_cur, attnT_cur, kc)

        # ---- steady: mm2(pair p) interleaved with mm1(pair p+1) ----
        for pr in range(NPAIR):
            if pr + 1 < NPAIR:
                x_nxt, xT_nxt = load_pair(pr + 1)
                attnT_nxt = atpool.tile([P, NKC, NQ], bf16, tag="attnT")
            else:
                x_nxt = xT_nxt = attnT_nxt = None
            if pr == 0:
                units = ([(qc, 0) for qc in range(NQF)]
                         + [(qc, 1) for qc in range(NQF)] + [("tail", 0)])
                # own chunks 5-8 during the s0 phase, pair-1 chunks after
                jobs = ([(xT_cur, attnT_cur, kc) for kc in range(5, 9)]
                        + [(xT_nxt, attnT_nxt, kc) for kc in range(NKC)])
                sched = [[0], [1], [2], [3], [4, 5], [6, 7], [8, 9], [10, 11], [12]]
            else:
                units = [(qc, s) for qc in range(NQF) for s in range(2)] + [("tail", 0)]
                jobs = ([(xT_nxt, attnT_nxt, kc) for kc in range(NKC)]
                        if attnT_nxt is not None else [])
                sched = [[i] if i < len(jobs) else [] for i in range(len(units))]
            ps_cur = None
            for i, (qc, s) in enumerate(units):
                if qc == "tail":
                    ps_cur = psS.tile([P, 2], f32, tag="ps")
                    mm2_tail(pr, x_cur, attnT_cur, ps_cur)
                else:
                    if s == 0 or pr == 0:
                        ps_cur = psS.tile([P, 2], f32, tag="ps")
                    mm2_full(pr, s, qc, x_cur, attnT_cur, ps_cur)
                for j in sched[i]:
                    mm1_chunk(*jobs[j])
            x_cur, xT_cur, attnT_cur = x_nxt, xT_nxt, attnT_nxt

    nc.compile()
    _BUILD_CACHE["nc"] = nc
    return nc


def make_in_maps(x, query, bias):
    # qT: [D, NQ] -> [p, dc, q] with d = dc*128 + p, scaled by QS, fp8
    qT_np = np.ascontiguousarray(
        (query.astype(np.float32) * QS).T.reshape(NDC, P, NQ).transpose(1, 0, 2)
    ).astype(ml_dtypes.float8_e4m3)
    # bias: [q, k] -> biasT [k, q] scaled by QS, duplicated along the pair
    # k axis, then [p, kc, q] with k_pair = kc*128 + p
    bT = bias.T.astype(np.float32) * QS
    bTp_np = np.ascontiguousarray(
        np.concatenate([bT, bT], axis=0).reshape(NKC, P, NQ).transpose(1, 0, 2)
    ).astype(ml_dtypes.bfloat16)
    x_bf = x.astype(ml_dtypes.bfloat16)
    x_f8 = x.astype(ml_dtypes.float8_e4m3)
    # xT kc-major pair-concatenated:
    # [B, k, d] -> pairs [B/2, 1152, d] -> [pair, p(d%128), kc, dc, k%128]
    xp = x_f8.reshape(B // 2, 2 * NQ, D)                  # [pairs, kp, d]
    xp = xp.reshape(B // 2, NKC, P, NDC, P)               # [pr, kc, k1, dc, p]
    xT_np = np.ascontiguousarray(xp.transpose(0, 4, 1, 3, 2))  # [pr, p, kc, dc, k1]
    in_maps = []
    for c in range(NCORES):
        in_maps.append({
            "xs": np.ascontiguousarray(x_bf[c * BPC:(c + 1) * BPC]),
            "xsT": xT_np[c * NPAIR:(c + 1) * NPAIR],
            "qT": qT_np,
            "bTp": bTp_np,
        })
    return in_maps


def kernel(x, query, bias):
    from concourse.bass_utils import run_bass_kernel_spmd

    nc = build_program()
    in_maps = make_in_maps(np.asarray(x), np.asarray(query), np.asarray(bias))
    res = run_bass_kernel_spmd(nc, in_maps, core_ids=list(range(NCORES)))
    return np.concatenate(
        [r["out"].astype(np.float32) for r in res.results], axis=0)


if __name__ == "__main__":
    rng = np.random.default_rng(0)
    x = rng.standard_normal((B, NQ, D), dtype=np.float32)
    q = rng.standard_normal((NQ, D), dtype=np.float32) / 32.0
    bias = 0.01 * rng.standard_normal((NQ, NQ), dtype=np.float32)
    o = kernel(x, q, bias)
    print(o.shape, o.dtype)
